# revision 13
# baseline (speedup 1.0000x reference)
"""MixedLoraLinear (base GEMM + segment-routed LoRA) on 8 TRN2 NeuronCores.

Strategy
--------
Token-shard across the 8 cores (1024 tokens each); replicate weights.
All routing (segment -> adapter -> scaling) is resolved on the host into a
dense [A*R, T] mask*scale matrix MT, so the device program is data-independent.

Per core we compute out^T [D_OUT, 1024] in bf16 compute / fp32 accumulate
(PSUM accumulation is fp32; bf16 inputs give rel err ~2e-3, well inside the
2e-2 gate, and halve both HBM traffic and LdWeights time vs fp32r):

  phase A:  hT[ar, t]   = sum_k WAcat[k, ar] * x[t, k]      (A*R = 128 rows)
            htm         = hT * MT_shard -> bf16              (mask+scale, DVE)
  phase B:  for each 128-row output block ob:
              psum[oo, t] = sum_k W[ob*128+oo, k] * x[t, k]  (32 k-steps)
                          + sum_ar WBcat[ar, ob*128+oo] * htm[ar, t]  (1 step)
              out = psum + bias  (ScalarE activation w/ per-partition bias)

Pipelining: phase A and the first two phase-B blocks are interleaved with the
streaming xt load (per-k-chunk DMAs), so the tensor engine starts within a few
us of kernel start instead of stalling ~58 us for the full x^T panel.  Weight
tiles are reused across both 512-token halves (th inner) to halve LdWeights.
wa, wb, mt, bias are SBUF-resident; base_w streams as 1 MB bf16 panels.
"""

import numpy as np
import ml_dtypes
from contextlib import ExitStack

import concourse.bass as bass
import concourse.tile as tile
from concourse import bacc, mybir
from concourse.bass_utils import run_bass_kernel_spmd

T, D_IN, D_OUT, R, A = 8192, 4096, 4096, 16, 8
N_CORES = 8
TOK = T // N_CORES          # 1024 tokens per core
KB = D_IN // 128            # 32 contraction blocks
OB = D_OUT // 128           # 32 output-row blocks
AR = A * R                  # 128 = one partition block
FREE = 512                  # matmul moving free dim (1 PSUM bank of fp32)
TH = TOK // FREE            # 2 token halves per core
NPRE = 2                    # phase-B blocks interleaved with the xt load

F32 = mybir.dt.float32
BF16 = mybir.dt.bfloat16


def _build_nc():
    nc = bacc.Bacc("TRN2", target_bir_lowering=False, debug=False,
                   num_devices=N_CORES)
    xt_d = nc.dram_tensor("xt", [128, KB * TOK], BF16, kind="ExternalInput").ap()
    wt_d = nc.dram_tensor("wt", [OB * 128, KB * 128], BF16, kind="ExternalInput").ap()
    wa_d = nc.dram_tensor("wa", [128, KB * AR], BF16, kind="ExternalInput").ap()
    wb_d = nc.dram_tensor("wb", [AR, D_OUT], BF16, kind="ExternalInput").ap()
    mt_d = nc.dram_tensor("mt", [AR, TOK], F32, kind="ExternalInput").ap()
    b_d = nc.dram_tensor("bias", [128, OB], F32, kind="ExternalInput").ap()
    out_d = nc.dram_tensor("outt", [D_OUT, TOK], BF16, kind="ExternalOutput").ap()

    with tile.TileContext(nc) as tc, ExitStack() as ctx:
        const = ctx.enter_context(tc.tile_pool(name="const", bufs=1))
        wt_pool = ctx.enter_context(tc.tile_pool(name="wt", bufs=4))
        out_pool = ctx.enter_context(tc.tile_pool(name="ot", bufs=4))
        psum_a = ctx.enter_context(tc.tile_pool(name="pa", bufs=1, space="PSUM"))
        psum_b = ctx.enter_context(tc.tile_pool(name="pb", bufs=3, space="PSUM"))

        def new_pb():
            # one [128, FREE] fp32 bank per token half, rotating over 3 bufs
            return [psum_b.tile([128, FREE], F32, tag=f"pb{th}", name=f"pb{th}")
                    for th in range(TH)]

        xt_sb = const.tile([128, KB * TOK], BF16)     # 64 KB/partition, resident
        wa_sb = const.tile([128, KB * AR], BF16)
        wb_sb = const.tile([AR, D_OUT], BF16)
        mt_sb = const.tile([AR, TOK], F32)
        htm_sb = const.tile([AR, TOK], BF16)
        b_sb = const.tile([128, OB], F32)

        # DMA issue order tracks first-use order so the tensor engine starts
        # within a few us: wa/xt chunk 0 land first, then the NPRE+1 phase-B
        # weight panels, then the remaining wa/xt stream (0.5 MB chunks to
        # bound the SyncE trigger backlog), then the post-loop operands.
        NCH = 16                       # xt/wa streamed in NCH chunks
        kper = KB // NCH               # k-blocks per chunk
        cw_x = kper * TOK
        cw_a = kper * AR
        wt_t = [wt_pool.tile([128, KB * 128], BF16, tag="wt", name=f"wt_t{i}")
                for i in range(NPRE + 1)]
        for ko in range(2):
            nc.sync.dma_start(wa_sb[:, ko * AR:(ko + 1) * AR],
                              wa_d[:, ko * AR:(ko + 1) * AR])
            nc.sync.dma_start(xt_sb[:, ko * TOK:(ko + 1) * TOK],
                              xt_d[:, ko * TOK:(ko + 1) * TOK])
        nc.sync.dma_start(wt_t[0][:], wt_d[0:128, :])
        for ko in range(2, 4):
            nc.sync.dma_start(wa_sb[:, ko * AR:(ko + 1) * AR],
                              wa_d[:, ko * AR:(ko + 1) * AR])
            nc.sync.dma_start(xt_sb[:, ko * TOK:(ko + 1) * TOK],
                              xt_d[:, ko * TOK:(ko + 1) * TOK])
        nc.sync.dma_start(wt_t[1][:], wt_d[128:256, :])
        for c in range(2, NCH):
            nc.sync.dma_start(wa_sb[:, c * cw_a:(c + 1) * cw_a],
                              wa_d[:, c * cw_a:(c + 1) * cw_a])
            nc.sync.dma_start(xt_sb[:, c * cw_x:(c + 1) * cw_x],
                              xt_d[:, c * cw_x:(c + 1) * cw_x])
        nc.sync.dma_start(wt_t[2][:], wt_d[256:384, :])
        nc.sync.dma_start(mt_sb[:], mt_d[:, :])
        nc.sync.dma_start(b_sb[:], b_d[:, :])
        nc.sync.dma_start(wb_sb[:], wb_d[:, :])

        # ---- startup: phase A + phase B blocks 0..NPRE-1, per k-chunk ----
        pa = [psum_a.tile([128, FREE], F32, tag=f"pa{th}", name=f"pa{th}")
              for th in range(TH)]
        pb_pre = [new_pb() for i in range(NPRE)]
        for ko in range(KB):
            xs = [xt_sb[:, ko * TOK + th * FREE: ko * TOK + (th + 1) * FREE]
                  for th in range(TH)]
            for th in range(TH):
                nc.tensor.matmul(pa[th][:],
                                 lhsT=wa_sb[:, ko * AR:(ko + 1) * AR],
                                 rhs=xs[th],
                                 start=(ko == 0), stop=(ko == KB - 1))
            for i in range(NPRE):
                for th in range(TH):
                    nc.tensor.matmul(pb_pre[i][th][:],
                                     lhsT=wt_t[i][:, ko * 128:(ko + 1) * 128],
                                     rhs=xs[th],
                                     start=(ko == 0), stop=False)

        # ---- mask+scale -> htm (bf16) ----
        for th in range(TH):
            nc.vector.tensor_mul(htm_sb[:, th * FREE:(th + 1) * FREE],
                                 pa[th][:], mt_sb[:, th * FREE:(th + 1) * FREE])

        def lora_first(ob, pb_th):
            """LoRA contribution as the first accumulation of the block."""
            for th in range(TH):
                nc.tensor.matmul(pb_th[th][:],
                                 lhsT=wb_sb[:, ob * 128:(ob + 1) * 128],
                                 rhs=htm_sb[:, th * FREE:(th + 1) * FREE],
                                 start=True, stop=False)

        def finish_block(ob, pb_th):
            """Bias + store for one output block (accumulation already done)."""
            # the very last stores go out in 256-col pieces so the final
            # activation/DMA tail is short
            npc = 2 if ob == OB - 1 else 1
            pc = FREE // npc
            for th in range(TH):
                ot = out_pool.tile([128, FREE], BF16)
                for p in range(npc):
                    nc.scalar.activation(ot[:, p * pc:(p + 1) * pc],
                                         pb_th[th][:, p * pc:(p + 1) * pc],
                                         mybir.ActivationFunctionType.Identity,
                                         bias=b_sb[:, ob:ob + 1])
                    nc.sync.dma_start(
                        out_d[ob * 128:(ob + 1) * 128,
                              th * FREE + p * pc: th * FREE + (p + 1) * pc],
                        ot[:, p * pc:(p + 1) * pc])

        for i in range(NPRE):
            for th in range(TH):
                nc.tensor.matmul(pb_pre[i][th][:],
                                 lhsT=wb_sb[:, i * 128:(i + 1) * 128],
                                 rhs=htm_sb[:, th * FREE:(th + 1) * FREE],
                                 start=False, stop=True)
            finish_block(i, pb_pre[i])

        # ---- steady state: remaining output blocks ----
        for ob in range(NPRE, OB):
            if ob == NPRE:
                wt_s = wt_t[NPRE]       # prefetched above
            else:
                wt_s = wt_pool.tile([128, KB * 128], BF16, tag="wt",
                                    name="wt_s")
                nc.sync.dma_start(wt_s[:], wt_d[ob * 128:(ob + 1) * 128, :])
            pb = new_pb()
            lora_first(ob, pb)
            for ko in range(KB):
                for th in range(TH):
                    nc.tensor.matmul(
                        pb[th][:],
                        lhsT=wt_s[:, ko * 128:(ko + 1) * 128],
                        rhs=xt_sb[:, ko * TOK + th * FREE:
                                  ko * TOK + (th + 1) * FREE],
                        start=False, stop=(ko == KB - 1))
            finish_block(ob, pb)
    nc.compile()
    return nc


_NC = None


def _get_nc():
    global _NC
    if _NC is None:
        _NC = _build_nc()
    return _NC


def _host_prep(x, base_w, base_b, wa, wb, scaling, segment, lora_ids):
    """Build the per-core input maps (bf16 weights/activations)."""
    x = np.asarray(x, np.float32)
    base_w = np.asarray(base_w, np.float32)
    base_b = np.asarray(base_b, np.float32)
    wa = np.asarray(wa, np.float32)
    wb = np.asarray(wb, np.float32)
    scaling = np.asarray(scaling, np.float32)
    segment = np.asarray(segment, np.int64)
    lora_ids = np.asarray(lora_ids, np.int64)

    # routing -> dense mask*scale [A*R, T]
    pos = np.arange(T)
    token_seg = np.clip(np.searchsorted(segment, pos, side="right") - 1, 0, A - 1)
    token_lora = lora_ids[token_seg]                      # [T]
    onehot = (token_lora[None, :] == np.arange(A)[:, None]).astype(np.float32)
    mt_full = np.repeat(onehot * scaling[:, None], R, axis=0)  # [A*R, T]
    mt_full = np.ascontiguousarray(mt_full)

    bf = ml_dtypes.bfloat16
    # weights (shared across cores)
    wt_pre = np.ascontiguousarray(
        base_w.reshape(OB, 128, KB, 128).transpose(0, 3, 2, 1)
        .reshape(OB * 128, KB * 128).astype(bf))
    wa_pre = np.ascontiguousarray(
        wa.transpose(1, 0, 2).reshape(KB, 128, AR).transpose(1, 0, 2)
        .reshape(128, KB * AR).astype(bf))
    wb_pre = np.ascontiguousarray(wb.reshape(AR, D_OUT).astype(bf))
    b_pre = np.ascontiguousarray(base_b.reshape(OB, 128).T)

    in_maps = []
    for c in range(N_CORES):
        xs = x[c * TOK:(c + 1) * TOK]                     # [TOK, D_IN]
        xt_pre = np.ascontiguousarray(
            xs.T.reshape(KB, 128, TOK).transpose(1, 0, 2)
            .reshape(128, KB * TOK).astype(bf))
        in_maps.append({
            "xt": xt_pre,
            "wt": wt_pre,
            "wa": wa_pre,
            "wb": wb_pre,
            "mt": np.ascontiguousarray(mt_full[:, c * TOK:(c + 1) * TOK]),
            "bias": b_pre,
        })
    return in_maps


def kernel(x, base_w, base_b, wa, wb, scaling, segment, lora_ids):
    in_maps = _host_prep(x, base_w, base_b, wa, wb, scaling, segment, lora_ids)
    nc = _get_nc()
    res = run_bass_kernel_spmd(nc, in_maps, core_ids=list(range(N_CORES)))
    parts = [np.asarray(res.results[c]["outt"], np.float32)
             for c in range(N_CORES)]                          # [D_OUT, TOK] each
    out_t = np.concatenate(parts, axis=1)                      # [D_OUT, T]
    return np.ascontiguousarray(out_t.T)                       # [T, D_OUT]
